# revision 14
# baseline (speedup 1.0000x reference)
"""FENet on 8 TRN2 cores — optimized bf16 composite-banded-matmul kernel.

Same composite strategy as the baseline (each feature f is
scale_f * sum |A_f @ x|, A_f host-built in fp64), plus:
  - per-chunk x-tile DMAs on the sync queue (block matmuls start as soon as
    their 128-position chunks land; weights stream on the gpsimd queue)
  - per-block weight DMA slices (tile-0 matmuls start before all weights)
  - tiny selector/scale DMAs issued first so sel matmuls never stall
  - per-row scales folded into the bf16 selector (rows normalized to max 1)
  - no on-chip transpose: output [8, B] per core, host transposes
  - 6 PSUM pz buffers + 2 pf banks for deep matmul runahead

fp8-e4m3 DoubleRow was evaluated and rejected: on real TRN2 a DR matmul
streams 1 column/cycle (same as bf16, despite CoreSim's 0.5 model), and the
2e-2 max-rel tolerance forces hi/lo splits of both w and x (90 matmuls/tile
vs 62 here), measured 167us vs 115us for this kernel.
"""

import os
import sys

import numpy as np

for _p in ("/opt/trn_rl_repo", os.path.expanduser("~/.axon_site/_ro/trn_rl_repo")):
    if os.path.isdir(_p) and _p not in sys.path:
        sys.path.insert(0, _p)

import concourse.bass as bass
import concourse.bacc as bacc
import concourse.mybir as mybir
from concourse import tile
from concourse.bass_utils import run_bass_kernel_spmd

F32 = mybir.dt.float32
BF16 = mybir.dt.bfloat16
NP_BF16 = mybir.dt.np(BF16)

N_CORES = 8
B_FULL = 24576
L_IN = 900
L_PAD = 1024
NCH = 8
B_LOC = B_FULL // N_CORES
N_TILE = 512
TILES = B_LOC // N_TILE

KER, STR, PAD_L, PAD_R = 40, 2, 38, 39
N_LAYERS = 7


def _conv_map(M, w):
    Mp = np.pad(M, ((PAD_L, PAD_R), (0, 0)))
    Lo = (Mp.shape[0] - KER) // STR + 1
    out = np.zeros((Lo, M.shape[1]), dtype=M.dtype)
    for k in range(KER):
        out += w[k] * Mp[k : k + STR * Lo : STR, :]
    return out


def _build_composite(feat_w, pass_w):
    P = np.eye(L_IN, dtype=np.float64)
    maps = []
    for i in range(N_LAYERS):
        F = _conv_map(P, feat_w[i, 0, 0].astype(np.float64))
        maps.append((F, 1.0 / F.shape[0]))
        P = _conv_map(P, pass_w[i, 0, 0].astype(np.float64))
    maps.append((P, 1.0 / 32.0))
    return maps


def _pack_blocks(maps):
    rows = []
    for fid, (A, sc) in enumerate(maps):
        for r in range(A.shape[0]):
            rows.append((fid, A[r]))
    n0 = maps[0][0].shape[0]
    n1 = maps[1][0].shape[0]
    groups = [rows[:n0], rows[n0 : n0 + n1], rows[n0 + n1 :]]
    blocks = []
    for g in groups:
        for s in range(0, len(g), 128):
            blk = g[s : s + 128]
            M = np.zeros((len(blk), L_PAD), dtype=np.float64)
            for r, (_, v) in enumerate(blk):
                M[r, :L_IN] = v
            chs = [c for c in range(NCH)
                   if np.any(M[:, c * 128 : (c + 1) * 128] != 0.0)]
            blocks.append(dict(M=M, chunks=chs, feats=[f for f, _ in blk]))
    return blocks


def _build_operands(blocks):
    n_mm = sum(len(b["chunks"]) for b in blocks)
    n_blk = len(blocks)
    wt = np.zeros((128, n_mm, 128), dtype=np.float32)
    sel = np.zeros((128, n_blk, 8), dtype=np.float32)
    sched = []
    g = 0
    for b, blk in enumerate(blocks):
        M = blk["M"]
        mrows = M.shape[0]
        amax = np.abs(M).max(axis=1)
        r = np.where(amax > 0, amax, 1.0)
        Mn = (M / r[:, None]).astype(np.float32)
        ent = []
        for c in blk["chunks"]:
            wt[:, g, :mrows] = Mn[:, c * 128 : (c + 1) * 128].T
            ent.append((g, c))
            g += 1
        for k, f in enumerate(blk["feats"]):
            sel[k, b, f] = r[k]
        sched.append(ent)
    return wt.astype(NP_BF16), sel.astype(NP_BF16), sched


def _build_program(sched, n_mm, n_blk):
    nc = bacc.Bacc()
    xs_d = nc.dram_tensor("xs", [128, TILES, NCH, N_TILE], BF16,
                          kind="ExternalInput")
    wt_d = nc.dram_tensor("wt", [128, n_mm, 128], BF16, kind="ExternalInput")
    sel_d = nc.dram_tensor("sel", [128, n_blk, 8], BF16, kind="ExternalInput")
    fs_d = nc.dram_tensor("fscale", [8, 1], F32, kind="ExternalInput")
    out_d = nc.dram_tensor("out", [8, B_LOC], F32, kind="ExternalOutput")

    with tile.TileContext(nc) as tc:
        with (
            tc.tile_pool(name="const", bufs=1) as constp,
            tc.tile_pool(name="xt", bufs=3) as xtp,
            tc.tile_pool(name="za", bufs=2) as zap,
            tc.tile_pool(name="oute", bufs=2) as outp,
            tc.tile_pool(name="pz", bufs=6, space=bass.MemorySpace.PSUM) as pzp,
            tc.tile_pool(name="pf", bufs=2, space=bass.MemorySpace.PSUM) as pfp,
        ):
            # tiny selector/scale tensors first (sync queue) so tile-0 sel
            # matmuls never wait behind the 1.7MB banded-weight stream, which
            # rides the gpsimd DGE queue
            sel_sb = constp.tile([128, n_blk, 8], BF16)
            nc.sync.dma_start(sel_sb[:], sel_d[:])
            fs_sb = constp.tile([8, 1], F32)
            nc.sync.dma_start(fs_sb[:], fs_d[:])
            wt_sb = constp.tile([128, n_mm, 128], BF16)
            for ent in sched:
                i0, i1 = ent[0][0], ent[-1][0] + 1
                nc.gpsimd.dma_start(wt_sb[:, i0:i1], wt_d[:, i0:i1])

            for t in range(TILES):
                trow = t * N_TILE
                xt = xtp.tile([128, NCH, N_TILE], BF16, tag="xt")
                # per-chunk DMA: block matmuls start once their chunks land
                for c in range(NCH):
                    nc.sync.dma_start(xt[:, c, :], xs_d[:, t, c, :])

                za = zap.tile([128, n_blk, N_TILE], BF16, tag="za")
                pf = pfp.tile([8, N_TILE], F32, tag="pf")
                for b, ent in enumerate(sched):
                    pz = pzp.tile([128, N_TILE], F32, tag="pz")
                    for j, (g, c) in enumerate(ent):
                        nc.tensor.matmul(
                            pz[:], wt_sb[:, g, :], xt[:, c, :],
                            start=(j == 0), stop=(j == len(ent) - 1),
                            skip_group_check=True)
                    nc.scalar.activation(
                        za[:, b, :], pz[:],
                        mybir.ActivationFunctionType.Abs)
                    nc.tensor.matmul(
                        pf[:], sel_sb[:, b, :], za[:, b, :],
                        start=(b == 0), stop=(b == n_blk - 1),
                        skip_group_check=True)

                fc = outp.tile([8, N_TILE], F32, tag="fc")
                nc.scalar.activation(
                    fc[:], pf[:], mybir.ActivationFunctionType.Copy,
                    scale=fs_sb[:])
                nc.gpsimd.dma_start(out_d[:, trow : trow + N_TILE], fc[:])
    nc.finalize()
    return nc


_CACHE = {}


def _get_program(feat_w, pass_w):
    maps = _build_composite(feat_w, pass_w)
    blocks = _pack_blocks(maps)
    wt, sel, sched = _build_operands(blocks)
    fscale = np.zeros((8, 1), dtype=np.float32)
    for fid, (A, sc) in enumerate(maps):
        fscale[fid, 0] = sc
    key = tuple(tuple(e) for e in sched)
    if key not in _CACHE:
        _CACHE[key] = _build_program(sched, wt.shape[1], sel.shape[1])
    return _CACHE[key], wt, sel, fscale


def _pack_x(x):
    xf = x.reshape(B_FULL, L_IN).astype(np.float32)
    xq = np.zeros((B_FULL, L_PAD), dtype=NP_BF16)
    xq[:, :L_IN] = xf.astype(NP_BF16)
    per_core = []
    for i in range(N_CORES):
        s = slice(i * B_LOC, (i + 1) * B_LOC)
        v = xq[s].reshape(TILES, N_TILE, NCH, 128).transpose(3, 0, 2, 1)
        per_core.append(np.ascontiguousarray(v))
    return per_core


def _make_inmaps(x, feat_w, pass_w):
    nc, wt, sel, fscale = _get_program(feat_w, pass_w)
    xs_cores = _pack_x(np.asarray(x, dtype=np.float32))
    return nc, [
        {"xs": xs_cores[i], "wt": wt, "sel": sel, "fscale": fscale}
        for i in range(N_CORES)
    ]


def kernel(x, feat_w, pass_w):
    nc, in_maps = _make_inmaps(x, feat_w, pass_w)
    res = run_bass_kernel_spmd(nc, in_maps, list(range(N_CORES)))
    out = np.concatenate([res.results[i]["out"] for i in range(N_CORES)],
                         axis=1)
    return np.ascontiguousarray(out.T.astype(np.float32))
